# revision 56
# baseline (speedup 1.0000x reference)
"""Trainium2 Bass kernel for LogisticRegressionRBF.

Computes sigmoid(exp(-||x_i - c_j||^2) @ w + b) for x [K, M], c [N, M],
w [N], b [1] with K = N = 8192, M = 64, sharded data-parallel over rows
of x across 8 NeuronCores.

Algorithm (per core, KS = K/8 = 1024 rows):
  - Host folds everything into one bf16 matmul via feature augmentation
    (67 features): with A = 2*log2(e)*2^23 and B = 127*2^23,
        xhat_k = [x_k, -||x_k||^2/2, 1, 1]
        chat_n = [A*c_n, A, A*(-||c_n||^2 + ln|w_n|)/2, B]
    so the PE produces P_kn = A*R_kn + B in PSUM, where
    2*R_kn = -||x_k - c_n||^2 + ln|w_n| and exp(2R) = |w_n| * phi_kn.
  - Basis columns are pre-sorted by sign(w) on the host (the n-sum is
    permutation invariant), so sum_n w_n phi_kn = S_pos - S_neg with
    each S a plain sum over a contiguous column range.
  - exp + row-sum of each 1024-column PSUM chunk runs on ONE of two
    engines, statically load-balanced ~59/41 so both stay saturated:
      * ACT: Exp(P*EXP_SCALE + EXP_BIAS) in place, with accum_out
        emitting the per-row partial sums for free (fused reduce);
      * DVE: Schraudolph bits — int32(max(P, 2^23)) IS the exp2 bit
        pattern; two pairwise folds on the otherwise-idle GPSIMD shrink
        the DVE bitcast-reduce to chunk/4.
  - A tiny DVE combine applies the +/- signs and adds b; sigmoid is one
    batched 0.5*tanh(z/2) + 0.5 at the end (tanh shares the ACT table
    set with exp — no table switch), then a single strided DMA out.
"""

import os
import sys
from contextlib import ExitStack

import numpy as np

try:
    import concourse.bass as bass  # noqa: F401
except ImportError:  # fresh grading dir: framework lives on these paths
    for _p in (
        "/root/.axon_site/_ro/trn_rl_repo",
        "/root/.axon_site/_ro/pypackages",
        "/opt/trn_rl_repo",
        "/opt/pypackages",
    ):
        if os.path.isdir(_p) and _p not in sys.path:
            sys.path.append(_p)
    import concourse.bass as bass  # noqa: F401

import concourse.tile as tile
from concourse import bacc, mybir
from concourse.bass_utils import run_bass_kernel_spmd

F32 = mybir.dt.float32
AF = mybir.ActivationFunctionType
ALU = mybir.AluOpType

N_CORES = 8
CHUNK = 1024  # exp-chunk granularity (PSUM tile columns)
PSUM_BUFS = 4
NT = 512      # matmul moving-operand free dim: 1 PSUM bank (fp32 max)

# Schraudolph exp2 bit-trick, folded into the matmul:
# basis features are pre-scaled by A = 2*log2(e)*2^23 and B = 127*2^23 is
# added via an extra augmented feature row, so PSUM holds P = A*R + B
# directly (where 2R = -||x-c||^2 + ln|w|, always << 0).
#   DVE path:  exp(2R) ~= bitcast_f32(int32(max(P, 2^23)))   (~3% rel err —
#     irrelevant here: every phi is ~1e-17 against an output of 0.5)
#   ACT path:  exp(2R) = Exp(P * EXP_SCALE + EXP_BIAS)  (exact)
# The lower clamp keeps the biased exponent >= 1 (no denormals, no
# negative-int garbage); P never overflows upward since 2R < ln(max|w|).
import ml_dtypes
EXP_A = float(np.float32(ml_dtypes.bfloat16(
    2.0 * 1.4426950408889634 * (1 << 23))))  # bf16-exact, used on host & chip
EXP_B = float(127 * (1 << 23))               # bf16-exact
EXP_CLAMP = float(1 << 23)                   # lower clamp on P
EXP_SCALE = float(np.float32(2.0 / EXP_A))
EXP_BIAS = float(np.float32(-EXP_B * (2.0 / EXP_A)))
# chunks with (global_chunk_idx % DVE_MOD) in DVE_PICK run on the DVE
# (~41% DVE / ~59% ACT — balances both engines; the spread was tuned
# against the cost-model timeline, with the tail biased toward ACT so
# the DVE backlog doesn't starve ACT at the end of the schedule)
DVE_MOD = 32
DVE_PICK = frozenset({1, 3, 6, 8, 11, 13, 16, 18, 19, 21, 23, 26, 28})


def set_config(chunk=None, psum_bufs=None, dve_frac=None):
    """Tune chunk size / psum buffering / DVE share (for config sweeps)."""
    global CHUNK, PSUM_BUFS, DVE_PICK
    if chunk is not None:
        CHUNK = chunk
    if psum_bufs is not None:
        PSUM_BUFS = psum_bufs
    if dve_frac is not None:
        count = max(0, min(DVE_MOD, round(dve_frac * DVE_MOD)))
        picks = set()
        i = 0
        while len(picks) < count:
            picks.add((1 + int(round(i * DVE_MOD / count))) % DVE_MOD)
            i += 1
        DVE_PICK = frozenset(picks)

LAST_RESULT = None  # BassKernelResults of the most recent run (for test.py)


def _plan_ranges(p_pos: int, n: int, chunk: int):
    """Sign-pure (lo, hi, sign) ranges per column chunk (chunk-relative)."""
    ranges = []
    for c0 in range(0, n, chunk):
        c1 = c0 + chunk
        if p_pos <= c0:
            ent = [(0, chunk, -1.0)]
        elif p_pos >= c1:
            ent = [(0, chunk, 1.0)]
        else:
            ent = [(0, p_pos - c0, 1.0), (p_pos - c0, chunk, -1.0)]
        ranges.append(ent)
    return ranges


def _build(nc, ks: int, n: int, c_dim: int, ranges, ncols: int,
           chunk: int, nt: int):
    BF16 = mybir.dt.bfloat16
    xT = nc.dram_tensor("xT", [c_dim, ks], BF16, kind="ExternalInput").ap()
    cT = nc.dram_tensor("cT", [c_dim, n], BF16, kind="ExternalInput").ap()
    sgn = nc.dram_tensor("sgn", [128, ncols], F32, kind="ExternalInput").ap()
    brep = nc.dram_tensor("brep", [128, 1], F32, kind="ExternalInput").ap()
    out = nc.dram_tensor("out", [ks, 1], F32, kind="ExternalOutput").ap()

    n_chunks = n // chunk
    n_ktiles = ks // 128

    with tile.TileContext(nc) as tc, ExitStack() as ctx:
        consts = ctx.enter_context(tc.tile_pool(name="consts", bufs=1))
        psum_pool = ctx.enter_context(
            tc.tile_pool(name="psum", bufs=PSUM_BUFS, space="PSUM"))
        spool = ctx.enter_context(tc.tile_pool(name="scols", bufs=3))
        small = ctx.enter_context(tc.tile_pool(name="small", bufs=4))
        dvework = ctx.enter_context(tc.tile_pool(name="dvework", bufs=3))

        # xT + the first cT chunk gate the first matmul — issue them first
        xT_sb = consts.tile([c_dim, ks], BF16, tag="xT_sb")
        nc.sync.dma_start(xT_sb[:], xT[:])
        cT_sb = consts.tile([c_dim, n], BF16, tag="cT_sb")
        # 2048-wide loads: halves the ~625ns/DMA HWDGE prep serialization
        # without delaying the first chunk too much (model optimum)
        for lo in range(0, n, 2 * chunk):
            hi = min(n, lo + 2 * chunk)
            nc.sync.dma_start(cT_sb[:, lo:hi], cT[:, lo:hi])
        sgn_sb = consts.tile([128, ncols], F32, tag="sgn_sb")
        nc.sync.dma_start(sgn_sb[:], sgn[:])
        b_sb = consts.tile([128, 1], F32, tag="b_sb")
        nc.sync.dma_start(b_sb[:], brep[:])
        ebias_sb = consts.tile([128, 1], F32, tag="ebias_sb")
        nc.vector.memset(ebias_sb[:], EXP_BIAS)

        I32 = mybir.dt.int32
        z_all = consts.tile([128, n_ktiles], F32, tag="z_all")
        res_all = consts.tile([128, n_ktiles], F32, tag="res_all")
        pending = []  # deferred DVE reduces: (src_ap, col_ap) — issued late
                      # so they don't head-of-line-block the DVE FIFO while
                      # the Pool folds run

        def flush_pending(upto):
            while len(pending) > upto:
                src, dst = pending.pop(0)
                nc.vector.reduce_sum(dst, src, axis=mybir.AxisListType.X)

        for kt in range(n_ktiles):
            lhsT = xT_sb[:, kt * 128:(kt + 1) * 128]
            scols = spool.tile([128, ncols], F32, tag="scols")
            col = 0
            for ch in range(n_chunks):
                ps = psum_pool.tile([128, chunk], F32, tag="ps")
                for q in range(chunk // nt):
                    nc.tensor.matmul(
                        ps[:, q * nt:(q + 1) * nt],
                        lhsT,
                        cT_sb[:, ch * chunk + q * nt: ch * chunk + (q + 1) * nt],
                        start=True, stop=True)
                gidx = kt * n_chunks + ch
                if (gidx % DVE_MOD) in DVE_PICK:
                    # DVE exp path (Schraudolph), frees the ACT engine
                    t2 = dvework.tile([128, chunk], I32, tag="t2")
                    nc.vector.tensor_scalar_max(t2[:], ps[:], EXP_CLAMP)
                    t2f = t2[:].bitcast(F32)
                    if len(ranges[ch]) == 1 and chunk % 4 == 0:
                        # sign-pure chunk: two pairwise folds on the idle
                        # Pool engine shrink the DVE reduce to chunk/4
                        h, q4 = chunk // 2, chunk // 4
                        f1 = dvework.tile([128, h], F32, tag="f1")
                        nc.gpsimd.tensor_add(f1[:], t2f[:, :h], t2f[:, h:])
                        f2 = dvework.tile([128, q4], F32, tag="f2")
                        nc.gpsimd.tensor_add(f2[:], f1[:, :q4], f1[:, q4:])
                        pending.append((f2[:], scols[:, col:col + 1]))
                        col += 1
                    else:
                        for (lo, hi, _s) in ranges[ch]:
                            pending.append(
                                (t2f[:, lo:hi], scols[:, col:col + 1]))
                            col += 1
                    flush_pending(2)
                else:
                    for (lo, hi, _s) in ranges[ch]:
                        nc.scalar.activation(
                            ps[:, lo:hi], ps[:, lo:hi], AF.Exp,
                            scale=EXP_SCALE, bias=ebias_sb[:],
                            accum_out=scols[:, col:col + 1])
                        col += 1
            flush_pending(0)
            assert col == ncols
            tmp = small.tile([128, ncols], F32, tag="tmp")
            nc.vector.tensor_mul(tmp[:], scols[:], sgn_sb[:])
            zs = small.tile([128, 1], F32, tag="zs")
            nc.vector.reduce_sum(zs[:], tmp[:], axis=mybir.AxisListType.X)
            nc.vector.tensor_scalar_add(z_all[:, kt:kt + 1], zs[:], b_sb[:])
        # one batched sigmoid tail: keeps tanh out of the ACT FIFO mid-stream
        th_all = consts.tile([128, n_ktiles], F32, tag="th_all")
        nc.scalar.activation(th_all[:], z_all[:], AF.Tanh, scale=0.5)
        nc.vector.tensor_scalar(res_all[:], th_all[:], 0.5, 0.5,
                                ALU.mult, ALU.add)
        out_view = out.rearrange("(a b) c -> b (a c)", b=128)
        nc.sync.dma_start(out_view, res_all[:])


def _prep(x, x_basis, w, b):
    """Host-side: sign-sort basis columns, build augmented transposed mats."""
    x = np.asarray(x, np.float32)
    xb = np.asarray(x_basis, np.float32)
    w = np.asarray(w, np.float32)
    b = np.asarray(b, np.float32)
    k, m = x.shape
    n = xb.shape[0]

    order = np.argsort(w < 0, kind="stable")  # w >= 0 first
    cs = xb[order]
    ws = w[order]
    p_pos = int((w >= 0).sum())
    with np.errstate(divide="ignore"):
        lw = np.where(ws == 0.0, -1e30, np.log(np.abs(ws, dtype=np.float64)))
    xsq = np.einsum("km,km->k", x, x, dtype=np.float64)
    csq = np.einsum("nm,nm->n", cs, cs, dtype=np.float64)

    xT = np.empty((m + 3, k), np.float32)
    xT[:m] = x.T
    xT[m] = -xsq / 2.0
    xT[m + 1] = 1.0
    xT[m + 2] = 1.0

    cT = np.empty((m + 3, n), np.float32)
    cT[:m] = cs.T * EXP_A
    cT[m] = EXP_A
    cT[m + 1] = EXP_A * (-csq + lw) / 2.0
    cT[m + 2] = EXP_B
    return xT, cT, p_pos, b


def host_setup(x, x_basis, w, b):
    """Everything host-side: returns (build_args, in_maps, dims)."""
    import ml_dtypes

    k, m = x.shape
    n = x_basis.shape[0]
    ks = k // N_CORES
    c_dim = m + 3

    xT, cT, p_pos, b32 = _prep(x, x_basis, w, b)
    ranges = _plan_ranges(p_pos, n, CHUNK)
    signs = [s for ent in ranges for (_lo, _hi, s) in ent]
    ncols = len(signs)
    sgn = np.tile(np.asarray(signs, np.float32)[None, :], (128, 1))
    brep = np.full((128, 1), float(b32[0]), np.float32)

    xT16 = xT.astype(ml_dtypes.bfloat16)
    cT16 = np.ascontiguousarray(cT.astype(ml_dtypes.bfloat16))
    in_maps = [
        {
            "xT": np.ascontiguousarray(xT16[:, cid * ks:(cid + 1) * ks]),
            "cT": cT16,
            "sgn": sgn,
            "brep": brep,
        }
        for cid in range(N_CORES)
    ]
    build_args = dict(ks=ks, n=n, c_dim=c_dim, ranges=ranges, ncols=ncols,
                      chunk=CHUNK, nt=NT)
    return build_args, in_maps


def kernel(x, x_basis, w, b):
    global LAST_RESULT
    build_args, in_maps = host_setup(x, x_basis, w, b)
    nc = bacc.Bacc("TRN2", target_bir_lowering=False, debug=False,
                   num_devices=N_CORES)
    _build(nc, **build_args)
    nc.compile()
    r = run_bass_kernel_spmd(
        nc, in_maps, list(range(N_CORES)),
        trace=bool(os.environ.get("BASS_KERNEL_TRACE")))
    LAST_RESULT = r
    return np.concatenate([r.results[i]["out"] for i in range(N_CORES)], 0)
